# revision 1
# baseline (speedup 1.0000x reference)
"""Trainium2 Bass kernel for equivariant multihead attention.

Math (per batch b, query point i, coset s1, channel c):
    logit[j,s2] = sum_g pairwise_g[b,i,j,s1,s2,g]*w_g[c,g]
                  + w_y[c,0]*y[b,j,s2,c] + w_y[c,1]*y[b,i,s1,c] + b_g[c] + b_y[c]
    att = exp(logit)*mask[b,j,s2];  att /= sum_{j,s2} att
    out = (y[b,i,s1,c] + sum_{j,s2} att*y[b,j,s2,c]) * mask[b,i,s1]  @ w_lin.T

The query-side term and the biases are constant over the key dims (j,s2), so
they cancel in the normalization and are dropped.  The key-side factor
exp(w_y[c,0]*y[b,j,s2,c])*mask[b,j,s2] is a tiny per-batch table KD (and
KD*y = KN), precomputed on host.  Per (b,i) block the device computes
    E[(s1,s2,c), j] = exp(sum_g G_T[(s1,s2,g), j] * w_g[c,g])
    den_part[(s1,s2,c)] = sum_j E * KD_T     (fused multiply-reduce)
    num_part[(s1,s2,c)] = sum_j E * KN_T
and one final PE matmul sums the partials over s2.  Host finishes with the
residual add, query mask, and the c_in->c_out linear (all tiny).

Sharding: query dim i is split 8 ways (16 i x 4 b = 64 blocks per core).

Implementation notes (walrus on this stack allows only ONE sync wait per
Matmult / DMA / STT instruction, and ~12 on the final drain):
  * ALL inputs ship as ONE dram "blob" per core, loaded by 7 big
    column-range DMAs into a single SBUF plane -> every DMA is the first
    on its HW-DGE queue (no proc-predecessor wait) and descriptor runs are
    ~17KB contiguous (max DMA efficiency).  The final store is the 8th DMA
    (queue 7, also virgin).
  * tiny "spacer" ops make each engine observe cross-engine ticks ahead of
    the real instructions, so those carry at most one wait each.
"""

import numpy as np

import concourse.bacc as bacc
import concourse.tile as tile
from concourse import mybir
from concourse.bass_utils import run_bass_kernel_spmd

B, N, S, CIN, COUT, GDIM = 4, 128, 8, 8, 8, 7
NCORES = 8
ISHARD = N // NCORES          # 16 query points per core
NBLK = B * ISHARD             # 64 (b,i) blocks per core
PW = S * GDIM                 # 56: free width of one s1 slice
QW = 2 * PW                   # 112: free width of one transpose quarter
NQ = 4                        # quarters per block
BW = NQ * QW                  # 448 floats per (j, block)
NCOL = NBLK * NQ              # 256 partial columns per half

# blob column layout: [ident | kd | kn | bd | sind | pg blocks]
IDENT0 = 0
KD0 = 128
KN0 = KD0 + B * N             # 640
BD0 = KN0 + B * N             # 1152
SIND0 = BD0 + 128             # 1280
CONSTW = SIND0 + 16           # 1296
TOTW = CONSTW + NBLK * BW     # 29968

# blocks covered by each of the 7 input DMAs (first also carries consts;
# earlier ones smaller for a faster pipeline ramp)
SUPER_BLOCKS = (4, 6, 8, 10, 11, 12, 13)

F32 = mybir.dt.float32

# per-quarter engine assignment (balance tuning): PSUM->SBUF copy of the
# transposed quarter, and den/num fused multiply-reduces.  bacc's
# split_sync_waits legalizes any multi-wait instructions this creates.
# (gpsimd cannot run scalar_tensor_tensor: not a valid Pool-engine opcode)
COPY_ENG = ("act", "act", "act", "dve")
DEN_ENG = ("dve", "dve", "dve", "dve")
NUM_ENG = ("dve", "dve", "dve", "dve")

_PROGRAM_CACHE = {}


def _build_program(nblk=NBLK, loop_reps=1):
    """loop_reps>1 wraps the main loop in a hardware For_i that re-runs the
    full pass (including the input DMAs) on the same data -- used only for
    timing: wall(loop_reps=R) - wall(loop_reps=1) isolates device time from
    the ~100ms axon dispatch/transfer overhead."""
    nc = bacc.Bacc("TRN2", target_bir_lowering=False, debug=False,
                   num_devices=NCORES)

    blob_d = nc.dram_tensor("blob", (N, TOTW), F32, kind="ExternalInput").ap()
    out_s = nc.dram_tensor("out_s", (16, 2 * NCOL), F32,
                           kind="ExternalOutput").ap()

    # per-super [start_block, end_block) and column ranges
    supers = []
    blk0 = 0
    for nb in SUPER_BLOCKS:
        if blk0 >= nblk:
            break
        nb = min(nb, nblk - blk0)
        c0 = 0 if blk0 == 0 else CONSTW + blk0 * BW
        c1 = CONSTW + (blk0 + nb) * BW
        supers.append((blk0, blk0 + nb, c0, c1))
        blk0 += nb

    with tile.TileContext(nc) as tc:
        with (
            tc.tile_pool(name="consts", bufs=1) as consts,
            tc.tile_pool(name="gtpool", bufs=4) as gtpool,
            tc.tile_pool(name="epool", bufs=4) as epool,
            tc.tile_pool(name="psA", bufs=4, space="PSUM") as psA,
            tc.tile_pool(name="psB", bufs=2, space="PSUM") as psB,
            tc.tile_pool(name="psC", bufs=1, space="PSUM") as psC,
        ):
            g_all = consts.tile([N, TOTW], F32)
            ident = g_all[:, IDENT0:IDENT0 + 128]
            bd = g_all[0:QW, BD0:BD0 + 128]
            sind = g_all[:, SIND0:SIND0 + 16]

            buf_dve = consts.tile([128, 2 * NCOL], F32)
            nc.vector.memset(buf_dve, 0.0)

            NDUM = 8
            dummies = [consts.tile([128, 1], F32, name=f"dum{i}")
                       for i in range(NDUM)]
            dum_idx = [0]
            s_sb = consts.tile([16, 2 * NCOL], F32)

            def stt_reduce(eng, e_q, table, col_ap):
                dum = dummies[dum_idx[0] % NDUM]
                dum_idx[0] += 1
                engine = nc.vector if eng == "dve" else nc.gpsimd
                engine.scalar_tensor_tensor(
                    dum.broadcast_to(e_q.shape), e_q, 0.0, table,
                    op0=mybir.AluOpType.bypass, op1=mybir.AluOpType.mult,
                    accum_out=col_ap)

            def main_pass():
              for (b0, b1, c0, c1) in supers:
                nc.sync.dma_start(g_all[:, c0:c1], blob_d[:, c0:c1])
              for (b0, b1, c0, c1) in supers:
                for blk in range(b0, b1):
                    b = blk // ISHARD
                    gcol = CONSTW + blk * BW
                    kd_b = g_all[:, KD0 + b * N:KD0 + (b + 1) * N]
                    kn_b = g_all[:, KN0 + b * N:KN0 + (b + 1) * N]

                    gt_cat = gtpool.tile([QW, NQ, 128], F32, tag="gt")
                    for q in range(NQ):
                        gt_ps = psA.tile([QW, 128], F32, tag="gtps")
                        nc.tensor.transpose(
                            gt_ps,
                            g_all[:, gcol + QW * q:gcol + QW * (q + 1)],
                            ident)
                        if COPY_ENG[q] == "act":
                            nc.scalar.copy(gt_cat[:, q, :], gt_ps)
                        else:
                            nc.vector.tensor_copy(gt_cat[:, q, :], gt_ps)

                    l_ps = psB.tile([128, NQ, 128], F32, tag="lps")
                    nc.tensor.matmul(l_ps, lhsT=bd, rhs=gt_cat,
                                     start=True, stop=True)

                    e_t = epool.tile([128, NQ, 128], F32, tag="e")
                    nc.scalar.activation(e_t, l_ps,
                                         mybir.ActivationFunctionType.Exp)

                    for q in range(NQ):
                        col = blk * NQ + q
                        e_q = e_t[:, q, :]
                        stt_reduce(DEN_ENG[q], e_q, kd_b,
                                   buf_dve[:, col:col + 1])
                        stt_reduce(NUM_ENG[q], e_q, kn_b,
                                   buf_dve[:, NCOL + col:NCOL + col + 1])

            if loop_reps > 1:
                with tc.For_i(0, loop_reps, 1,
                              hint_engines=(mybir.EngineType.PE,
                                            mybir.EngineType.Activation,
                                            mybir.EngineType.DVE,
                                            mybir.EngineType.SP)):
                    main_pass()
            else:
                main_pass()

            # sum the (h,s2,c) j-partials over s2 -> (h,c)
            s_ps = psC.tile([16, 2 * NCOL], F32)
            nc.tensor.matmul(s_ps, lhsT=sind, rhs=buf_dve,
                             start=True, stop=True)
            nc.scalar.copy(s_sb, s_ps)
            nc.sync.dma_start(out_s, s_sb)   # 8th DMA -> virgin queue 7

    nc.compile()   # bacc: register alloc + split_sync_waits (1-wait limit)
    return nc


def _get_program(nblk=NBLK, loop_reps=1):
    key = ("nc", nblk, loop_reps)
    if key not in _PROGRAM_CACHE:
        _PROGRAM_CACHE[key] = _build_program(nblk, loop_reps)
    return _PROGRAM_CACHE[key]


def _host_prep(pairwise_g, coset_functions, mask, w_y, w_g):
    """Build the per-core input blobs."""
    y = coset_functions.astype(np.float32)          # (B, N, S, C) keys
    maskf = mask.astype(np.float32)
    ey = np.exp(y * w_y[:, 0]) * maskf[..., None]   # (B, j, s2, c)
    kn = ey * y
    # rows (h, s2, c) with h in {0,1} duplicated; cols j
    kd_t = np.tile(ey.transpose(0, 2, 3, 1).reshape(B, S * CIN, N), (1, 2, 1))
    kn_t = np.tile(kn.transpose(0, 2, 3, 1).reshape(B, S * CIN, N), (1, 2, 1))

    bd = np.zeros((128, 128), np.float32)
    for pl in range(16):
        for g in range(GDIM):
            for c in range(CIN):
                bd[pl * GDIM + g, pl * CIN + c] = w_g[c, g]

    sind = np.zeros((128, 16), np.float32)
    for h in range(2):
        for s2 in range(S):
            for c in range(CIN):
                sind[h * 64 + s2 * CIN + c, h * CIN + c] = 1.0

    consts_plane = np.empty((N, CONSTW), np.float32)
    consts_plane[:, IDENT0:IDENT0 + 128] = np.eye(128, dtype=np.float32)
    consts_plane[:, KD0:KD0 + B * N] = kd_t.transpose(1, 0, 2).reshape(128, -1)
    consts_plane[:, KN0:KN0 + B * N] = kn_t.transpose(1, 0, 2).reshape(128, -1)
    consts_plane[:, BD0:BD0 + 128] = bd
    consts_plane[:, SIND0:SIND0 + 16] = sind

    in_maps = []
    for k in range(NCORES):
        sl = slice(ISHARD * k, ISHARD * (k + 1))
        pg_core = pairwise_g[:, sl].reshape(NBLK, N, BW)
        blob = np.empty((N, TOTW), np.float32)
        blob[:, :CONSTW] = consts_plane
        blob[:, CONSTW:] = pg_core.transpose(1, 0, 2).reshape(N, NBLK * BW)
        in_maps.append({"blob": blob})
    return in_maps


def _host_finish(s_list, coset_functions, mask, w_lin):
    """Decode per-core (16, 512) outputs into the full result."""
    y = np.asarray(coset_functions, dtype=np.float32)
    maskf = np.asarray(mask).astype(np.float32)
    out = np.empty((B, N, S, COUT), np.float32)
    for k in range(NCORES):
        s = s_list[k]
        den = s[:, :NCOL].reshape(2, CIN, NBLK, NQ)
        num = s[:, NCOL:].reshape(2, CIN, NBLK, NQ)
        # (h, c, blk, q) -> (blk, s1 = 2q + h, c)
        den = den.transpose(2, 3, 0, 1).reshape(NBLK, S, CIN)
        num = num.transpose(2, 3, 0, 1).reshape(NBLK, S, CIN)
        sl = slice(ISHARD * k, ISHARD * (k + 1))
        y_q = y[:, sl].reshape(NBLK, S, CIN)
        m_q = maskf[:, sl].reshape(NBLK, S)
        res = (y_q + num / den) * m_q[..., None]
        res = res @ w_lin.T
        out[:, sl] = res.reshape(B, ISHARD, S, COUT)
    return out


def kernel(pairwise_g, coset_functions, mask, w_y, b_y, w_g, b_g, w_lin):
    pairwise_g = np.asarray(pairwise_g, dtype=np.float32)
    coset_functions = np.asarray(coset_functions, dtype=np.float32)
    mask = np.asarray(mask)
    w_y = np.asarray(w_y, dtype=np.float32)
    w_g = np.asarray(w_g, dtype=np.float32)
    w_lin = np.asarray(w_lin, dtype=np.float32)

    nc = _get_program()
    in_maps = _host_prep(pairwise_g, coset_functions, mask, w_y, w_g)
    res = run_bass_kernel_spmd(nc, in_maps, core_ids=list(range(NCORES)))
    s_list = [r["out_s"] for r in res.results]
    return _host_finish(s_list, coset_functions, mask, w_lin)



# revision 8
# speedup vs baseline: 4.1275x; 4.1275x over previous
"""Trainium2 Bass kernel for equivariant multihead attention (v2).

Math (per batch b, query point i, coset s1, channel c):
    logit[j,s2] = sum_g pairwise_g[b,i,j,s1,s2,g]*w_g[c,g]
                  + w_y[c,0]*y[b,j,s2,c] + w_y[c,1]*y[b,i,s1,c] + b_g[c] + b_y[c]
    att = exp(logit)*mask[b,j,s2];  att /= sum_{j,s2} att
    out = (y[b,i,s1,c] + sum_{j,s2} att*y[b,j,s2,c]) * mask[b,i,s1]  @ w_lin.T

Query-side terms and biases are constant over the key dims (j,s2) and cancel
in the normalization -> dropped.  The key-side factor is folded into the
exponent: lkd[c,(s2,j)] = w_y[c,0]*y[b,j,s2,c] + log(mask) (0 / -80).

Layout trick: one PE matmul per PAIR of query blocks computes the complete
biased logits.  PSUM rows m = (blk2, s1, c) = 128, free cols = (s2, j) = 1024
keys.  Contraction rows k = (blk2, s1, g) = 112 rows of pairwise data plus
8 indicator rows that add lkd[c] -> K = 120.  Then
    E = exp(PSUM)              (ACT, one instr; accum_out gives den for free)
    num = sum_keys E * y_tab   (DVE, one scalar_tensor_tensor with accum)
and den/num land in two columns of a [128, 64] accumulator that is DMA'd out
once.  Host finishes with residual add, query mask and the c_in->c_out linear.

All matmul operands are bf16 (tolerance is 2e-2; bf16 keeps PE at 1 cyc/row
and halves HBM traffic); exp/accumulations are f32.

Sharding: query dim i is split 8 ways (16 i x 4 b = 64 blocks = 32 units/core).
"""

import numpy as np
import ml_dtypes

import concourse.bacc as bacc
import concourse.tile as tile
from concourse import mybir
from concourse.bass_utils import run_bass_kernel_spmd

B, N, S, CIN, COUT, GDIM = 4, 128, 8, 8, 8, 7
NCORES = 8
ISHARD = N // NCORES          # 16 query points per core
NBLK = B * ISHARD             # 64 (b,i) blocks per core
NUNIT = NBLK // 2             # 32 units of 2 blocks
KEYW = S * N                  # 1024 key columns, s2-major: col = s2*128 + j
HALF = KEYW // 2              # one PSUM bank of f32
KROWS = 2 * S * GDIM + CIN    # 120 contraction rows (112 pairwise + 8 lkd)

# bf16 blob column layout: [ y_tab | w2 | unit tiles ]
YT0 = 0
W20 = B * KEYW                # 4096
U0 = W20 + 128                # 4224
W16 = U0 + NUNIT * KEYW       # 36992

# DMA schedule after the first (W2 + units 0-2) DMA: ("u", a, b) loads unit
# tiles [a, b), ("y", a, b) loads y_tab batches [a, b).  Sized so each
# transfer's completion lands just before compute needs it (compute drains
# ~1.2us/unit; DMA delivers ~0.8us/unit serially).
DMA_ITEMS = (("y", 0, 1), ("u", 2, 4), ("u", 4, 7), ("u", 7, 11),
             ("y", 1, 2), ("u", 11, 16), ("u", 16, 22), ("y", 2, 3),
             ("u", 22, 28), ("y", 3, 4), ("u", 28, 32))

F32 = mybir.dt.float32
BF16 = mybir.dt.bfloat16
NPBF16 = ml_dtypes.bfloat16

_PROGRAM_CACHE = {}


def _build_program(nblk=NBLK, loop_reps=1):
    """loop_reps>1 wraps the main loop in a hardware For_i that re-runs the
    full pass (including the input DMAs) on the same data -- used only for
    timing: wall(loop_reps=R) - wall(loop_reps=1) isolates device time from
    the ~100ms axon dispatch/transfer overhead."""
    nunit = nblk // 2
    nc = bacc.Bacc("TRN2", target_bir_lowering=False, debug=False,
                   num_devices=NCORES)

    blob_d = nc.dram_tensor("blob16", (128, W16), BF16,
                            kind="ExternalInput").ap()
    out_s = nc.dram_tensor("out_s", (128, 2 * NUNIT), F32,
                           kind="ExternalOutput").ap()

    items = [it for it in DMA_ITEMS
             if it[0] == "y" or it[1] < nunit]
    items = [(t, a, min(b_, nunit)) if t == "u" else (t, a, b_)
             for (t, a, b_) in items]

    with tile.TileContext(nc) as tc:
        with (
            tc.tile_pool(name="consts", bufs=1) as consts,
            tc.tile_pool(name="epool", bufs=3) as epool,
            tc.tile_pool(name="psum", bufs=3, space="PSUM") as psum,
        ):
            g16 = consts.tile([128, W16], BF16)
            w2 = g16[0:KROWS, W20:W20 + 128]
            acc = consts.tile([128, 2 * NUNIT], F32)

            NDUM = 8
            dummies = [consts.tile([128, 1], BF16, name=f"dum{i}")
                       for i in range(NDUM)]

            def main_pass():
                nc.sync.dma_start(g16[0:KROWS, W20:U0 + 2 * KEYW],
                                  blob_d[0:KROWS, W20:U0 + 2 * KEYW])
                for (t, a, b_) in items:
                    if t == "u":
                        c0, c1 = U0 + a * KEYW, U0 + b_ * KEYW
                        nc.sync.dma_start(g16[0:KROWS, c0:c1],
                                          blob_d[0:KROWS, c0:c1])
                    else:
                        c0, c1 = a * KEYW, b_ * KEYW
                        nc.sync.dma_start(g16[:, c0:c1], blob_d[:, c0:c1])
                for u in range(nunit):
                    b = u // (nunit // B)
                    ucol = U0 + u * KEYW
                    pt = psum.tile([128, KEYW], F32, tag="l")
                    for h in range(2):
                        nc.tensor.matmul(
                            pt[:, h * HALF:(h + 1) * HALF],
                            lhsT=w2,
                            rhs=g16[0:KROWS,
                                    ucol + h * HALF:ucol + (h + 1) * HALF],
                            start=True, stop=True)
                    e_t = epool.tile([128, KEYW], BF16, tag="e")
                    nc.scalar.activation(e_t, pt,
                                         mybir.ActivationFunctionType.Exp,
                                         accum_out=acc[:, u:u + 1])
                    dum = dummies[u % NDUM]
                    nc.vector.scalar_tensor_tensor(
                        dum.broadcast_to(e_t.shape), e_t, 0.0,
                        g16[:, YT0 + b * KEYW:YT0 + (b + 1) * KEYW],
                        op0=mybir.AluOpType.bypass, op1=mybir.AluOpType.mult,
                        accum_out=acc[:, NUNIT + u:NUNIT + u + 1])

            if loop_reps > 1:
                with tc.For_i(0, loop_reps, 1,
                              hint_engines=(mybir.EngineType.PE,
                                            mybir.EngineType.Activation,
                                            mybir.EngineType.DVE,
                                            mybir.EngineType.SP)):
                    main_pass()
            else:
                main_pass()

            nc.sync.dma_start(out_s, acc)

    nc.compile()   # bacc: register alloc + split_sync_waits (1-wait limit)
    return nc


def _get_program(nblk=NBLK, loop_reps=1):
    key = ("nc", nblk, loop_reps)
    if key not in _PROGRAM_CACHE:
        _PROGRAM_CACHE[key] = _build_program(nblk, loop_reps)
    return _PROGRAM_CACHE[key]


def _host_prep(pairwise_g, coset_functions, mask, w_y, w_g):
    """Build the per-core bf16 input blobs."""
    y = coset_functions.astype(np.float32)          # (B, N, S, C) keys
    logmask = np.where(np.asarray(mask, bool), 0.0, -80.0).astype(np.float32)

    # lkd[b, c, (s2, j)] = w_y[c,0]*y[b,j,s2,c] + logmask[b,j,s2]
    yT = y.transpose(0, 3, 2, 1)                    # (B, C, S, N) = [b,c,s2,j]
    lkd = w_y[:, 0][None, :, None, None] * yT \
        + logmask.transpose(0, 2, 1)[:, None, :, :]
    lkd = lkd.reshape(B, CIN, KEYW)

    # y_tab[p, b*KEYW + (s2,j)] = y[b, j, s2, p % C]
    ytab = np.tile(yT.reshape(B, CIN, KEYW), (1, 128 // CIN, 1))
    ytab128 = ytab.transpose(1, 0, 2).reshape(128, B * KEYW)

    # W2 [KROWS, 128]: col m = (blk2, s1, c)
    w2 = np.zeros((KROWS, 128), np.float32)
    for blk2 in range(2):
        for s1 in range(S):
            for g in range(GDIM):
                for c in range(CIN):
                    w2[blk2 * 56 + s1 * GDIM + g,
                       blk2 * 64 + s1 * CIN + c] = w_g[c, g]
    for blk2 in range(2):
        for s1 in range(S):
            for c in range(CIN):
                w2[2 * S * GDIM + c, blk2 * 64 + s1 * CIN + c] = 1.0

    in_maps = []
    for k in range(NCORES):
        sl = slice(ISHARD * k, ISHARD * (k + 1))
        pg = pairwise_g[:, sl]                      # (B, 16, N, S, S, G)
        pgr = pg.reshape(B, 8, 2, N, S, S, GDIM)    # [b, iu, blk2, j, s1, s2, g]
        pgr = pgr.transpose(0, 1, 2, 4, 6, 5, 3)    # [b, iu, blk2, s1, g, s2, j]
        pgr = pgr.reshape(B, 8, 112, KEYW)
        units = np.empty((B, 8, KROWS, KEYW), np.float32)
        units[:, :, :112] = pgr
        units[:, :, 112:] = lkd[:, None, :, :]
        units = units.reshape(NUNIT, KROWS, KEYW)

        blob = np.zeros((128, W16), NPBF16)
        blob[:, YT0:YT0 + B * KEYW] = ytab128
        blob[0:KROWS, W20:W20 + 128] = w2
        blob[0:KROWS, U0:] = units.transpose(1, 0, 2).reshape(KROWS, -1)
        in_maps.append({"blob16": blob})
    return in_maps


def _host_finish(s_list, coset_functions, mask, w_lin):
    """Decode per-core (128, 64) den|num columns into the full result."""
    y = np.asarray(coset_functions, dtype=np.float32)
    maskf = np.asarray(mask).astype(np.float32)
    out = np.empty((B, N, S, COUT), np.float32)
    for k in range(NCORES):
        s = np.asarray(s_list[k], np.float32)
        den = s[:, :NUNIT].reshape(2, S, CIN, B, 8)   # [blk2, s1, c, b, iu]
        num = s[:, NUNIT:].reshape(2, S, CIN, B, 8)
        den = den.transpose(3, 4, 0, 1, 2).reshape(B, ISHARD, S, CIN)
        num = num.transpose(3, 4, 0, 1, 2).reshape(B, ISHARD, S, CIN)
        sl = slice(ISHARD * k, ISHARD * (k + 1))
        y_q = y[:, sl]
        m_q = maskf[:, sl]
        res = (y_q + num / den) * m_q[..., None]
        out[:, sl] = res @ w_lin.T
    return out


def kernel(pairwise_g, coset_functions, mask, w_y, b_y, w_g, b_g, w_lin):
    pairwise_g = np.asarray(pairwise_g, dtype=np.float32)
    coset_functions = np.asarray(coset_functions, dtype=np.float32)
    mask = np.asarray(mask)
    w_y = np.asarray(w_y, dtype=np.float32)
    w_g = np.asarray(w_g, dtype=np.float32)
    w_lin = np.asarray(w_lin, dtype=np.float32)

    nc = _get_program()
    in_maps = _host_prep(pairwise_g, coset_functions, mask, w_y, w_g)
    res = run_bass_kernel_spmd(nc, in_maps, core_ids=list(range(NCORES)))
    s_list = [r["out_s"] for r in res.results]
    return _host_finish(s_list, coset_functions, mask, w_lin)


# revision 11
# speedup vs baseline: 4.7941x; 1.1615x over previous
"""Trainium2 Bass kernel for equivariant multihead attention (v2).

Math (per batch b, query point i, coset s1, channel c):
    logit[j,s2] = sum_g pairwise_g[b,i,j,s1,s2,g]*w_g[c,g]
                  + w_y[c,0]*y[b,j,s2,c] + w_y[c,1]*y[b,i,s1,c] + b_g[c] + b_y[c]
    att = exp(logit)*mask[b,j,s2];  att /= sum_{j,s2} att
    out = (y[b,i,s1,c] + sum_{j,s2} att*y[b,j,s2,c]) * mask[b,i,s1]  @ w_lin.T

Query-side terms and biases are constant over the key dims (j,s2) and cancel
in the normalization -> dropped.  The key-side factor is folded into the
exponent: lkd[c,(s2,j)] = w_y[c,0]*y[b,j,s2,c] + log(mask) (0 / -80).

Layout trick: one PE matmul per PAIR of query blocks computes the complete
biased logits.  PSUM rows m = (blk2, s1, c) = 128, free cols = (s2, j) = 1024
keys.  Contraction rows k = (blk2, s1, g) = 112 rows of pairwise data plus
8 indicator rows that add lkd[c] -> K = 120.  Then
    E = exp(PSUM)              (ACT, one instr; accum_out gives den for free)
    num = sum_keys E * y_tab   (DVE, one scalar_tensor_tensor with accum)
and den/num land in two columns of a [128, 64] accumulator that is DMA'd out
once.  Host finishes with residual add, query mask and the c_in->c_out linear.

All matmul operands are bf16 (tolerance is 2e-2; bf16 keeps PE at 1 cyc/row
and halves HBM traffic); exp/accumulations are f32.

Sharding: query dim i is split 8 ways (16 i x 4 b = 64 blocks = 32 units/core).
"""

import numpy as np
import ml_dtypes

import concourse.bacc as bacc
import concourse.tile as tile
from concourse import mybir
from concourse.bass_utils import run_bass_kernel_spmd

B, N, S, CIN, COUT, GDIM = 4, 128, 8, 8, 8, 7
NCORES = 8
ISHARD = N // NCORES          # 16 query points per core
NBLK = B * ISHARD             # 64 (b,i) blocks per core
NUNIT = NBLK // 2             # 32 units of 2 blocks
KEYW = S * N                  # 1024 key columns, s2-major: col = s2*128 + j
HALF = KEYW // 2              # one PSUM bank of f32
KROWS = 2 * S * GDIM + CIN    # 120 contraction rows (112 pairwise + 8 lkd)

# bf16 blob column layout: [ y_tab | w2 | unit tiles ]
YT0 = 0
W20 = B * KEYW                # 4096
U0 = W20 + 128                # 4224
W16 = U0 + NUNIT * KEYW       # 36992

# DMA schedule after the first (W2 + units 0-2) DMA: ("u", a, b) loads unit
# tiles [a, b), ("y", a, b) loads y_tab batches [a, b).  Sized so each
# transfer's completion lands just before compute needs it (compute drains
# ~1.2us/unit; DMA delivers ~0.8us/unit serially).
DMA_ITEMS = (("u", 1, 2), ("y", 0, 1), ("u", 2, 4), ("u", 4, 7),
             ("u", 7, 11), ("y", 1, 2), ("u", 11, 16), ("u", 16, 22),
             ("y", 2, 3), ("u", 22, 28), ("y", 3, 4), ("u", 28, 32))

F32 = mybir.dt.float32
BF16 = mybir.dt.bfloat16
NPBF16 = ml_dtypes.bfloat16

_PROGRAM_CACHE = {}


def _build_program(nblk=NBLK, loop_reps=1):
    """loop_reps>1 wraps the main loop in a hardware For_i that re-runs the
    full pass (including the input DMAs) on the same data -- used only for
    timing: wall(loop_reps=R) - wall(loop_reps=1) isolates device time from
    the ~100ms axon dispatch/transfer overhead."""
    nunit = nblk // 2
    nc = bacc.Bacc("TRN2", target_bir_lowering=False, debug=False,
                   num_devices=NCORES)

    blob_d = nc.dram_tensor("blob16", (128, W16), BF16,
                            kind="ExternalInput").ap()
    out_s = nc.dram_tensor("out_s", (128, 2 * NUNIT), F32,
                           kind="ExternalOutput").ap()

    items = [it for it in DMA_ITEMS
             if it[0] == "y" or it[1] < nunit]
    items = [(t, a, min(b_, nunit)) if t == "u" else (t, a, b_)
             for (t, a, b_) in items]
    first_units = 1

    with tile.TileContext(nc) as tc:
        with (
            tc.tile_pool(name="consts", bufs=1) as consts,
            tc.tile_pool(name="epool", bufs=3) as epool,
            tc.tile_pool(name="psum", bufs=3, space="PSUM") as psum,
        ):
            g16 = consts.tile([128, W16], BF16)
            w2 = g16[0:KROWS, W20:W20 + 128]
            acc = consts.tile([128, 2 * NUNIT], F32)

            NDUM = 8
            dummies = [consts.tile([128, 1], BF16, name=f"dum{i}")
                       for i in range(NDUM)]

            def main_pass():
                nc.sync.dma_start(g16[0:KROWS, W20:U0 + first_units * KEYW],
                                  blob_d[0:KROWS, W20:U0 + first_units * KEYW])
                for (t, a, b_) in items:
                    if t == "u":
                        c0, c1 = U0 + a * KEYW, U0 + b_ * KEYW
                        nc.sync.dma_start(g16[0:KROWS, c0:c1],
                                          blob_d[0:KROWS, c0:c1])
                    else:
                        c0, c1 = a * KEYW, b_ * KEYW
                        nc.sync.dma_start(g16[:, c0:c1], blob_d[:, c0:c1])
                for u in range(nunit):
                    b = u // (nunit // B)
                    ucol = U0 + u * KEYW
                    pt = psum.tile([128, KEYW], F32, tag="l")
                    for h in range(2):
                        nc.tensor.matmul(
                            pt[:, h * HALF:(h + 1) * HALF],
                            lhsT=w2,
                            rhs=g16[0:KROWS,
                                    ucol + h * HALF:ucol + (h + 1) * HALF],
                            start=True, stop=True)
                    e_t = epool.tile([128, KEYW], BF16, tag="e")
                    nc.scalar.activation(e_t, pt,
                                         mybir.ActivationFunctionType.Exp,
                                         accum_out=acc[:, u:u + 1])
                    dum = dummies[u % NDUM]
                    nc.vector.scalar_tensor_tensor(
                        dum.broadcast_to(e_t.shape), e_t, 0.0,
                        g16[:, YT0 + b * KEYW:YT0 + (b + 1) * KEYW],
                        op0=mybir.AluOpType.bypass, op1=mybir.AluOpType.mult,
                        accum_out=acc[:, NUNIT + u:NUNIT + u + 1])

            if loop_reps > 1:
                with tc.For_i(0, loop_reps, 1,
                              hint_engines=(mybir.EngineType.PE,
                                            mybir.EngineType.Activation,
                                            mybir.EngineType.DVE,
                                            mybir.EngineType.SP)):
                    main_pass()
            else:
                main_pass()

            nc.sync.dma_start(out_s, acc)

    nc.compile()   # bacc: register alloc + split_sync_waits (1-wait limit)
    return nc


def _get_program(nblk=NBLK, loop_reps=1):
    key = ("nc", nblk, loop_reps)
    if key not in _PROGRAM_CACHE:
        _PROGRAM_CACHE[key] = _build_program(nblk, loop_reps)
    return _PROGRAM_CACHE[key]


def _host_prep(pairwise_g, coset_functions, mask, w_y, w_g):
    """Build the per-core bf16 input blobs."""
    y = coset_functions.astype(np.float32)          # (B, N, S, C) keys
    logmask = np.where(np.asarray(mask, bool), 0.0, -80.0).astype(np.float32)

    # lkd[b, c, (s2, j)] = w_y[c,0]*y[b,j,s2,c] + logmask[b,j,s2]
    yT = y.transpose(0, 3, 2, 1)                    # (B, C, S, N) = [b,c,s2,j]
    lkd = w_y[:, 0][None, :, None, None] * yT \
        + logmask.transpose(0, 2, 1)[:, None, :, :]
    lkd = lkd.reshape(B, CIN, KEYW)

    # y_tab[p, b*KEYW + (s2,j)] = y[b, j, s2, p % C]
    ytab = np.tile(yT.reshape(B, CIN, KEYW), (1, 128 // CIN, 1))
    ytab128 = ytab.transpose(1, 0, 2).reshape(128, B * KEYW)

    # W2 [KROWS, 128]: col m = (blk2, s1, c)
    w2 = np.zeros((KROWS, 128), np.float32)
    for blk2 in range(2):
        for s1 in range(S):
            for g in range(GDIM):
                for c in range(CIN):
                    w2[blk2 * 56 + s1 * GDIM + g,
                       blk2 * 64 + s1 * CIN + c] = w_g[c, g]
    for blk2 in range(2):
        for s1 in range(S):
            for c in range(CIN):
                w2[2 * S * GDIM + c, blk2 * 64 + s1 * CIN + c] = 1.0

    in_maps = []
    for k in range(NCORES):
        sl = slice(ISHARD * k, ISHARD * (k + 1))
        pg = pairwise_g[:, sl]                      # (B, 16, N, S, S, G)
        pgr = pg.reshape(B, 8, 2, N, S, S, GDIM)    # [b, iu, blk2, j, s1, s2, g]
        pgr = pgr.transpose(0, 1, 2, 4, 6, 5, 3)    # [b, iu, blk2, s1, g, s2, j]
        pgr = pgr.reshape(B, 8, 112, KEYW)
        units = np.empty((B, 8, KROWS, KEYW), np.float32)
        units[:, :, :112] = pgr
        units[:, :, 112:] = lkd[:, None, :, :]
        units = units.reshape(NUNIT, KROWS, KEYW)

        blob = np.zeros((128, W16), NPBF16)
        blob[:, YT0:YT0 + B * KEYW] = ytab128
        blob[0:KROWS, W20:W20 + 128] = w2
        blob[0:KROWS, U0:] = units.transpose(1, 0, 2).reshape(KROWS, -1)
        in_maps.append({"blob16": blob})
    return in_maps


def _host_finish(s_list, coset_functions, mask, w_lin):
    """Decode per-core (128, 64) den|num columns into the full result."""
    y = np.asarray(coset_functions, dtype=np.float32)
    maskf = np.asarray(mask).astype(np.float32)
    out = np.empty((B, N, S, COUT), np.float32)
    for k in range(NCORES):
        s = np.asarray(s_list[k], np.float32)
        den = s[:, :NUNIT].reshape(2, S, CIN, B, 8)   # [blk2, s1, c, b, iu]
        num = s[:, NUNIT:].reshape(2, S, CIN, B, 8)
        den = den.transpose(3, 4, 0, 1, 2).reshape(B, ISHARD, S, CIN)
        num = num.transpose(3, 4, 0, 1, 2).reshape(B, ISHARD, S, CIN)
        sl = slice(ISHARD * k, ISHARD * (k + 1))
        y_q = y[:, sl]
        m_q = maskf[:, sl]
        res = (y_q + num / den) * m_q[..., None]
        out[:, sl] = res @ w_lin.T
    return out


def kernel(pairwise_g, coset_functions, mask, w_y, b_y, w_g, b_g, w_lin):
    pairwise_g = np.asarray(pairwise_g, dtype=np.float32)
    coset_functions = np.asarray(coset_functions, dtype=np.float32)
    mask = np.asarray(mask)
    w_y = np.asarray(w_y, dtype=np.float32)
    w_g = np.asarray(w_g, dtype=np.float32)
    w_lin = np.asarray(w_lin, dtype=np.float32)

    nc = _get_program()
    in_maps = _host_prep(pairwise_g, coset_functions, mask, w_y, w_g)
    res = run_bass_kernel_spmd(nc, in_maps, core_ids=list(range(NCORES)))
    s_list = [r["out_s"] for r in res.results]
    return _host_finish(s_list, coset_functions, mask, w_lin)
